# revision 41
# baseline (speedup 1.0000x reference)
"""nd-GQA attention kernel for 8 Trainium2 NeuronCores.

Math reformulation (exact): the reference einsum 'bghsd,bhad->bhsa' SUMS the
group axis g before softmax, and RoPE is linear in x for fixed coords, so
  q_eff = q @ Wq_eff  with Wq_eff[i,h,d] = sum_g Wq[i, h*G+g, d]
turns the problem into plain 4-head attention.  The interleaved-pair RoPE is
converted to rotate-half form by permuting the head-dim columns of Wq_eff/Wk
(pairs (2i,2i+1) -> (i, 32+i)); V/Wo stay unpermuted, so scores (inner product
over d, invariant under a shared permutation) and outputs are unchanged.

Sharding: core c handles batch b = c//2 and effective-head pair hp = c%2
(tensor parallel over kv heads, Wo row-split).  Each core returns a partial
[S,1024] output; the host sums the two partials per batch (the all-reduce).

On-device layout: feature-major q/k (dim on partitions, seq on free axis),
token-major v.  Scores are computed transposed ([kv, sq] in PSUM) so the
PV matmul consumes exp(scores) directly with no transposes; a ones-column
appended to V yields the softmax denominators as PSUM row 64.
"""

import math

import numpy as np

B, S, IN_D = 4, 2048, 1024
HD, G = 64, 4          # head dim, query groups
H_EFF = 4              # effective kv heads after group-sum
N_CORES = 8

TWO_PI = 2.0 * math.pi
MAGIC = 12582912.0     # 1.5 * 2**23: float32 round-to-nearest-int trick
# 2*pi split for Cody-Waite range reduction (c1/c2 have short mantissas so
# k*c1 and k*c2 are exact for k < 2^14)
CW_C1 = 6.28125
CW_C2 = 1.9350051879882812e-03
CW_C3 = 3.0198032640827773e-07

_cache = {}


def _build_program(sq, skv, dbg=False):
    import concourse.bass as bass
    import concourse.mybir as mybir
    import concourse.tile as tile
    from concourse import bacc, bass_isa
    from concourse.bass import ts
    from concourse.masks import make_identity

    f16, f32, f32r = mybir.dt.float16, mybir.dt.float32, mybir.dt.float32r
    AF = mybir.ActivationFunctionType
    Alu = mybir.AluOpType

    IO = IN_D // 128       # input-dim chunks
    AO = skv // 128        # kv-position chunks
    SB = 512               # sq block for attention phase
    NB_Q, NB_K = sq // 512, skv // 512

    nc = bacc.Bacc("TRN2", target_bir_lowering=False, debug=False,
                   num_devices=N_CORES)

    qx = nc.dram_tensor("qx", [sq, IN_D], f16, kind="ExternalInput").ap()
    kvx = nc.dram_tensor("kvx", [skv, IN_D], f16, kind="ExternalInput").ap()
    qcm = nc.dram_tensor("qcm", [128, sq], f32, kind="ExternalInput").ap()
    kcm = nc.dram_tensor("kcm", [128, skv], f32, kind="ExternalInput").ap()
    wq = nc.dram_tensor("wq", [IN_D, 128], f16, kind="ExternalInput").ap()
    wk = nc.dram_tensor("wk", [IN_D, 128], f16, kind="ExternalInput").ap()
    wv = nc.dram_tensor("wv", [IN_D, 128], f16, kind="ExternalInput").ap()
    wo0 = nc.dram_tensor("wo0", [64, IN_D], f16, kind="ExternalInput").ap()
    wo1 = nc.dram_tensor("wo1", [64, IN_D], f16, kind="ExternalInput").ap()
    ifm = nc.dram_tensor("ifm", [128, 1], f32, kind="ExternalInput").ap()
    sgn = nc.dram_tensor("sgn", [128, 1], f32, kind="ExternalInput").ap()
    out = nc.dram_tensor("out", [sq, IN_D], f32, kind="ExternalOutput").ap()
    dbg_t = {}
    if dbg:
        for nm, shp, dt_ in [("d_ang", [128, skv], f32), ("d_m", [128, skv], f32),
                             ("d_sin", [128, skv], f32), ("d_cos", [128, skv], f32),
                             ("d_krT", [128, skv], f16), ("d_qrT", [128, sq], f16),
                             ("d_vaug", [128, (skv // 128) * 130], f16),
                             ("d_pvn0", [64, sq], f16), ("d_pvn1", [64, sq], f16),
                             ("d_ex", [128, 2 * 512], f16),
                             ("d_kraw", [128, 512], f32),
                             ("d_pv0", [65, 512], f32),
                             ("d_rec", [128, 512], f32),
                             ("d_rbc", [64, 512], f32),
                             ("d_u", [128, 512], f16),
                             ("d_t", [128, 512], f16),
                             ("d_tsw", [128, 512], f16)]:
            dbg_t[nm] = nc.dram_tensor(nm, shp, dt_, kind="ExternalOutput").ap()

    with tile.TileContext(nc) as tc:
        with (
            tc.tile_pool(name="persist", bufs=1) as persist,
            tc.tile_pool(name="wide", bufs=1) as wide,
        ):
            qrT = persist.tile([128, sq], f16)    # rope'd q, feature-major
            krT = persist.tile([128, skv], f16)   # rope'd k, feature-major
            v_aug = persist.tile([128, AO, 130], f16)  # [v0|1|v1|1] token-major
            pvn0 = persist.tile([64, sq], f16)    # normalized attn@v, head 0
            pvn1 = persist.tile([64, sq], f16)
            wq_sb = persist.tile([128, IO, 128], f16)
            wk_sb = persist.tile([128, IO, 128], f16)
            wv_sb = persist.tile([128, IO, 128], f16)
            wo0_sb = persist.tile([64, IN_D], f16)
            wo1_sb = persist.tile([64, IN_D], f16)
            ifm_sb = persist.tile([128, 1], f32)
            sgn_sb = persist.tile([128, 1], f32)
            nbias = persist.tile([128, 1], f32)   # exp shift constant
            nc.vector.memset(nbias, -3.0)

            nc.sync.dma_start(wq_sb, wq.rearrange("(io ii) d -> ii io d", ii=128))
            nc.sync.dma_start(wk_sb, wk.rearrange("(io ii) d -> ii io d", ii=128))
            nc.sync.dma_start(wv_sb, wv.rearrange("(io ii) d -> ii io d", ii=128))
            nc.gpsimd.dma_start(wo0_sb, wo0)
            nc.gpsimd.dma_start(wo1_sb, wo1)
            nc.gpsimd.dma_start(ifm_sb, ifm)
            nc.gpsimd.dma_start(sgn_sb, sgn)
            nc.vector.memset(v_aug[:, :, 64:65], 1.0)
            nc.vector.memset(v_aug[:, :, 129:130], 1.0)

            # ---- Phase 1: load transposed activations, rope tables,
            #      projections + rope, v projection ----
            with (
                tc.tile_pool(name="acts", bufs=1) as acts,
                tc.tile_pool(name="scr", bufs=1) as scr,
                tc.tile_pool(name="rope", bufs=2) as rope,
                tc.tile_pool(name="ps_pr", bufs=2, space="PSUM") as ps_pr,
            ):
                qT = acts.tile([128, IO, sq], f16, tag="qT")
                kvT = acts.tile([128, IO, skv], f16, tag="kvT")
                qx_r = qx.rearrange("s (io ii) -> s io ii", ii=128)
                kvx_r = kvx.rearrange("s (io ii) -> s io ii", ii=128)
                ident = acts.tile([128, 128], f16, tag="ident")
                make_identity(nc, ident)
                for io in range(IO):
                    nc.sync.dma_start(out=kvT[:, io], in_=kvx_r[:, io], transpose=True)
                    nc.sync.dma_start(out=qT[:, io], in_=qx_r[:, io], transpose=True)
                    # HAM warm-up: keep the PE busy while input transposes
                    # stream in, so projections start at full clock.  Each
                    # dummy matmul depends on one transpose, spreading them
                    # across the DMA window.
                    wps = ps_pr.tile([128, 512], f32, tag="warm")
                    nc.tensor.matmul(wps, lhsT=kvT[:, io, 0:128],
                                     rhs=kvT[:, io, 0:512], start=True, stop=True)
                    wps2 = ps_pr.tile([128, 512], f32, tag="warm")
                    nc.tensor.matmul(wps2, lhsT=qT[:, io, 0:128],
                                     rhs=qT[:, io, 0:512], start=True, stop=True)

                for side, (s_len, xT, w_sb, out_rT) in enumerate([
                    (skv, kvT, wk_sb, krT),
                    (sq, qT, wq_sb, qrT),
                ]):
                    coord_dram = kcm if side == 0 else qcm
                    cm = scr.tile([128, s_len], f32, tag=f"cm{side}")
                    nc.gpsimd.dma_start(cm, coord_dram)

                    # angles[p, s] = coord[axis(p), s] * inv_freq(p)
                    ang = scr.tile([128, s_len], f32, tag="ang")
                    nc.vector.tensor_scalar_mul(ang, cm, ifm_sb[:])
                    # range reduction: m = ang - round(ang/2pi)*2pi in [-pi, pi]
                    kq = scr.tile([128, s_len], f32, tag="kq")
                    nc.vector.tensor_scalar(kq, ang, 1.0 / TWO_PI, MAGIC,
                                            Alu.mult, Alu.add)
                    nc.vector.tensor_scalar(kq, kq, MAGIC, None, Alu.subtract)
                    m = scr.tile([128, s_len], f32, tag="m")
                    nc.vector.cody_waite_cascade(m, ang, kq, CW_C1, CW_C2, CW_C3)
                    carg = scr.tile([128, s_len], f32, tag="carg")
                    nc.vector.add_range_wrap(carg, m, math.pi / 2, math.pi, TWO_PI)
                    # sinF[p] = sign(p)*sin(f_p) (sign folded via per-partition
                    # scale), cosF[p] = cos(f_p)
                    sinF = scr.tile([128, s_len], f32, tag="sinF")
                    nc.scalar.activation(sinF, m, AF.Sin, scale=sgn_sb[:, 0:1])
                    cosF = scr.tile([128, s_len], f32, tag="cosF")
                    nc.scalar.activation(cosF, carg, AF.Sin)
                    if dbg and side == 0:
                        nc.gpsimd.dma_start(dbg_t["d_ang"], ang)
                        nc.gpsimd.dma_start(dbg_t["d_m"], m)
                        nc.gpsimd.dma_start(dbg_t["d_sin"], sinF)
                        nc.gpsimd.dma_start(dbg_t["d_cos"], cosF)

                    # projection + rope, 512 columns at a time
                    for blk in range(s_len // 512):
                        raw = ps_pr.tile([128, 512], f32, tag="raw")
                        for io in range(IO):
                            nc.tensor.matmul(raw, lhsT=w_sb[:, io],
                                             rhs=xT[:, io, ts(blk, 512)],
                                             start=(io == 0), stop=(io == IO - 1))
                        u = rope.tile([128, 512], f16, tag="u")
                        nc.vector.tensor_tensor(u, raw, cosF[:, ts(blk, 512)], Alu.mult)
                        t = rope.tile([128, 512], f16, tag="t")
                        nc.vector.tensor_tensor(t, raw, sinF[:, ts(blk, 512)], Alu.mult)
                        tsw = rope.tile([128, 512], f16, tag="tsw")
                        for hb in range(2):     # swap 32-row halves per head
                            lo, hi = hb * 64, hb * 64 + 32
                            nc.gpsimd.dma_start(out=tsw[lo:lo + 32], in_=t[hi:hi + 32])
                            nc.gpsimd.dma_start(out=tsw[hi:hi + 32], in_=t[lo:lo + 32])
                        if dbg and side == 0 and blk == 0:
                            stg_raw = rope.tile([128, 512], f32, tag="dbgraw")
                            nc.vector.tensor_copy(stg_raw, raw)
                            nc.gpsimd.dma_start(dbg_t["d_kraw"], stg_raw)
                            nc.gpsimd.dma_start(dbg_t["d_u"], u)
                            nc.gpsimd.dma_start(dbg_t["d_t"], t)
                            nc.gpsimd.dma_start(dbg_t["d_tsw"], tsw)
                        nc.vector.tensor_tensor(out_rT[:, ts(blk, 512)], u, tsw, Alu.add)

                # v projection: feature-major vT = Wv.T @ kv.T (stationary Wv,
                # N=512 moving), then PE-transpose 128x128 chunks into the
                # token-major augmented V
                for blk in range(skv // 512):
                    vps = ps_pr.tile([128, 512], f32, tag="raw")
                    for io in range(IO):
                        nc.tensor.matmul(vps, lhsT=wv_sb[:, io],
                                         rhs=kvT[:, io, ts(blk, 512)],
                                         start=(io == 0), stop=(io == IO - 1))
                    vT_sb = rope.tile([128, 512], f16, tag="vt")
                    nc.vector.tensor_copy(vT_sb, vps)
                    for t_i in range(4):
                        ao = blk * 4 + t_i
                        vp = ps_pr.tile([128, 128], f16, tag="vp")
                        nc.tensor.transpose(vp, vT_sb[:, ts(t_i, 128)], ident)
                        dst = v_aug[:, ao].rearrange("p (h x) -> p h x", x=65)[:, :, 0:64]
                        nc.vector.tensor_copy(dst, vp.rearrange("p (h d) -> p h d", d=64))

                if dbg:
                    nc.gpsimd.dma_start(dbg_t["d_krT"], krT)
                    nc.gpsimd.dma_start(dbg_t["d_qrT"], qrT)
                    nc.gpsimd.dma_start(dbg_t["d_vaug"],
                                        v_aug.rearrange("p a c -> p (a c)"))

            # ---- Phase 2: attention (scores^T -> exp -> PV+denominator),
            #      then normalize ----
            with (
                tc.tile_pool(name="ps_sc", bufs=2, space="PSUM") as ps_sc,
                tc.tile_pool(name="ps_pv", bufs=2, space="PSUM") as ps_pv,
                tc.tile_pool(name="att", bufs=4) as att,
                tc.tile_pool(name="nrm", bufs=2) as nrm,
            ):
                for sb in range(sq // SB):
                    pv0 = ps_pv.tile([65, SB], f32, tag="pv0")
                    pv1 = ps_pv.tile([65, SB], f32, tag="pv1")
                    for ao in range(AO):
                        sc = ps_sc.tile([128, 2, 512], f32, tag="sc")
                        nc.tensor.matmul(sc[:, 0], lhsT=krT[0:64, ts(ao, 128)],
                                         rhs=qrT[0:64, ts(sb, SB)],
                                         start=True, stop=True, tile_position=(0, 0))
                        nc.tensor.matmul(sc[:, 1], lhsT=krT[64:128, ts(ao, 128)],
                                         rhs=qrT[64:128, ts(sb, SB)],
                                         start=True, stop=True, tile_position=(64, 0))
                        ex = att.tile([128, 2, 512], f16, tag="ex")
                        nc.scalar.activation(ex, sc, AF.Exp, bias=nbias[:], scale=0.125)
                        if dbg and sb == 0 and ao == 0:
                            nc.gpsimd.dma_start(dbg_t["d_ex"],
                                                ex.rearrange("p a b -> p (a b)"))
                        nc.tensor.matmul(pv0, lhsT=v_aug[:, ao, 0:65], rhs=ex[:, 0],
                                         start=(ao == 0), stop=(ao == AO - 1))
                        nc.tensor.matmul(pv1, lhsT=v_aug[:, ao, 65:130], rhs=ex[:, 1],
                                         start=(ao == 0), stop=(ao == AO - 1))
                    for h, pv, pvn in ((0, pv0, pvn0), (1, pv1, pvn1)):
                        den = nrm.tile([128, SB], f32, tag="den")
                        nc.vector.memset(den, 0.0)
                        nc.vector.tensor_copy(den[64:65], pv[64:65])
                        dbc = nrm.tile([128, SB], f32, tag="dbc")
                        nc.gpsimd.partition_all_reduce(
                            dbc, den, channels=128, reduce_op=bass_isa.ReduceOp.add)
                        rbc = nrm.tile([64, SB], f32, tag="rbc")
                        nc.vector.reciprocal_approx_fast(out=rbc, in_=dbc[0:64])
                        if dbg and sb == 0 and h == 0:
                            stg_pv = nrm.tile([65, SB], f32, tag="dbgpv")
                            nc.vector.tensor_copy(stg_pv, pv)
                            nc.gpsimd.dma_start(dbg_t["d_pv0"], stg_pv)
                            nc.gpsimd.dma_start(dbg_t["d_rec"], den)
                            nc.gpsimd.dma_start(dbg_t["d_rbc"], rbc)
                        nc.vector.tensor_tensor(pvn[:, ts(sb, SB)], pv[0:64], rbc,
                                                Alu.mult)

                    # output projection for this s-block (row-split Wo ->
                    # partial sums; host adds the pair of cores per batch).
                    # lhsT changes once per psum tile to amortize LDWEIGHTS.
                    for sc_i in range(SB // 128):
                        g = sb * (SB // 128) + sc_i
                        op = ps_sc.tile([128, 2, 512], f32, tag="sc")
                        nc.tensor.matmul(op[:, 0], lhsT=pvn0[:, ts(g, 128)],
                                         rhs=wo0_sb[:, 0:512], start=True, stop=False)
                        nc.tensor.matmul(op[:, 1], lhsT=pvn0[:, ts(g, 128)],
                                         rhs=wo0_sb[:, 512:1024], start=True, stop=False)
                        nc.tensor.matmul(op[:, 0], lhsT=pvn1[:, ts(g, 128)],
                                         rhs=wo1_sb[:, 0:512], start=False, stop=True)
                        nc.tensor.matmul(op[:, 1], lhsT=pvn1[:, ts(g, 128)],
                                         rhs=wo1_sb[:, 512:1024], start=False, stop=True)
                        stg = att.tile([128, 2, 512], f32, tag="stg")
                        nc.any.tensor_copy(stg, op)
                        nc.gpsimd.dma_start(out[ts(g, 128), :],
                                            stg.rearrange("p a b -> p (a b)"))

                if dbg:
                    nc.gpsimd.dma_start(dbg_t["d_pvn0"], pvn0)
                    nc.gpsimd.dma_start(dbg_t["d_pvn1"], pvn1)

    nc.compile()
    return nc


def _get_program(sq, skv, dbg=False):
    key = (sq, skv, dbg)
    if key not in _cache:
        _cache[key] = _build_program(sq, skv, dbg=dbg)
    return _cache[key]


def _host_prep(q, q_coords, kv, kv_coords, Wq, Wk, Wv, Wo, sq, skv):
    q = np.ascontiguousarray(np.asarray(q, dtype=np.float32))
    kv = np.ascontiguousarray(np.asarray(kv, dtype=np.float32))
    qc = np.asarray(q_coords).astype(np.float32)
    kc = np.asarray(kv_coords).astype(np.float32)
    Wq = np.asarray(Wq, dtype=np.float32)
    Wk = np.asarray(Wk, dtype=np.float32)
    Wv = np.asarray(Wv, dtype=np.float32)
    Wo = np.asarray(Wo, dtype=np.float32)

    new_order = np.concatenate([np.arange(0, HD, 2), np.arange(1, HD, 2)])
    Wq_eff = Wq.reshape(IN_D, H_EFF, G, HD).sum(axis=2)
    Wq_p = Wq_eff[:, :, new_order].astype(np.float16)   # [IN, 4, 64]
    Wk_p = Wk[:, :, new_order].astype(np.float16)
    Wv16 = Wv.astype(np.float16)
    Wo16 = Wo.astype(np.float16)                         # [256, 1024]
    q16 = q.astype(np.float16)
    kv16 = kv.astype(np.float16)

    inv_freq = (10000.0 ** (-np.arange(16, dtype=np.float64) * (2.0 / 32))
                ).astype(np.float32)
    p = np.arange(128)
    f = (p % 64) % 32
    axis_of_p = f // 16
    j = f % 16
    ifm = inv_freq[j].astype(np.float32).reshape(128, 1)
    sgn = np.where((p % 64) < 32, 1.0, -1.0).astype(np.float32).reshape(128, 1)

    in_maps = []
    for core in range(N_CORES):
        b, hp = core // 2, core % 2
        in_maps.append({
            "qx": np.ascontiguousarray(q16[b, :sq]),
            "kvx": np.ascontiguousarray(kv16[b, :skv]),
            # coordinate of the rope axis used by each partition row
            "qcm": np.ascontiguousarray(qc[b, :sq][:, axis_of_p].T),
            "kcm": np.ascontiguousarray(kc[b, :skv][:, axis_of_p].T),
            "wq": np.ascontiguousarray(Wq_p[:, 2 * hp:2 * hp + 2].reshape(IN_D, 128)),
            "wk": np.ascontiguousarray(Wk_p[:, 2 * hp:2 * hp + 2].reshape(IN_D, 128)),
            "wv": np.ascontiguousarray(Wv16[:, 2 * hp:2 * hp + 2].reshape(IN_D, 128)),
            "wo0": np.ascontiguousarray(Wo16[hp * 128:hp * 128 + 64]),
            "wo1": np.ascontiguousarray(Wo16[hp * 128 + 64:hp * 128 + 128]),
            "ifm": ifm,
            "sgn": sgn,
        })
    return in_maps


def run_sharded(q, q_coords, kv, kv_coords, Wq, Wk, Wv, Wo, sq=S, skv=S,
                trace=False, dbg=False):
    from concourse import bass_utils

    nc = _get_program(sq, skv, dbg=dbg)
    in_maps = _host_prep(q, q_coords, kv, kv_coords, Wq, Wk, Wv, Wo, sq, skv)
    res = bass_utils.run_bass_kernel_spmd(
        nc, in_maps, core_ids=list(range(N_CORES)), trace=trace)
    parts = [r["out"] for r in res.results]
    out = np.stack([parts[2 * b] + parts[2 * b + 1] for b in range(B)])
    return out.astype(np.float32), res


def kernel(q, q_coords, kv, kv_coords, Wq, Wk, Wv, Wo):
    out, _ = run_sharded(q, q_coords, kv, kv_coords, Wq, Wk, Wv, Wo)
    return out


# revision 44
# speedup vs baseline: 1.2311x; 1.2311x over previous
"""nd-GQA attention kernel for 8 Trainium2 NeuronCores.

Math reformulation (exact): the reference einsum 'bghsd,bhad->bhsa' SUMS the
group axis g before softmax, and RoPE is linear in x for fixed coords, so
  q_eff = q @ Wq_eff  with Wq_eff[i,h,d] = sum_g Wq[i, h*G+g, d]
turns the problem into plain 4-head attention.  The interleaved-pair RoPE is
converted to rotate-half form by permuting the head-dim columns of Wq_eff/Wk
(pairs (2i,2i+1) -> (i, 32+i)); V/Wo stay unpermuted, so scores (inner product
over d, invariant under a shared permutation) and outputs are unchanged.

Sharding: core c handles batch b = c//2 and effective-head pair hp = c%2
(tensor parallel over kv heads, Wo row-split).  Each core returns a partial
[S,1024] output; the host sums the two partials per batch (the all-reduce).

On-device layout: feature-major q/k (dim on partitions, seq on free axis),
token-major v.  Scores are computed transposed ([kv, sq] in PSUM) so the
PV matmul consumes exp(scores) directly with no transposes; a ones-column
appended to V yields the softmax denominators as PSUM row 64.
"""

import math

import ml_dtypes
import numpy as np

BF16 = ml_dtypes.bfloat16

B, S, IN_D = 4, 2048, 1024
HD, G = 64, 4          # head dim, query groups
H_EFF = 4              # effective kv heads after group-sum
N_CORES = 8

TWO_PI = 2.0 * math.pi
MAGIC = 12582912.0     # 1.5 * 2**23: float32 round-to-nearest-int trick
# 2*pi split for Cody-Waite range reduction (c1/c2 have short mantissas so
# k*c1 and k*c2 are exact for k < 2^14)
CW_C1 = 6.28125
CW_C2 = 1.9350051879882812e-03
CW_C3 = 3.0198032640827773e-07

_cache = {}


def _build_program(sq, skv, dbg=False):
    import concourse.bass as bass
    import concourse.mybir as mybir
    import concourse.tile as tile
    from concourse import bacc, bass_isa
    from concourse.bass import ts
    from concourse.masks import make_identity

    # NB: "f16" binds to bfloat16 — fp16 matmuls stream at 2 cycles/row on
    # TRN2 (only bf16/fp8 run at 1 cycle/row), measured 427ns vs 213ns per
    # N=512 matmul.
    f16, f32, f32r = mybir.dt.bfloat16, mybir.dt.float32, mybir.dt.float32r
    AF = mybir.ActivationFunctionType
    Alu = mybir.AluOpType

    IO = IN_D // 128       # input-dim chunks
    AO = skv // 128        # kv-position chunks
    SB = 512               # sq block for attention phase
    NB_Q, NB_K = sq // 512, skv // 512

    nc = bacc.Bacc("TRN2", target_bir_lowering=False, debug=False,
                   num_devices=N_CORES)

    qx = nc.dram_tensor("qx", [sq, IN_D], f16, kind="ExternalInput").ap()
    kvx = nc.dram_tensor("kvx", [skv, IN_D], f16, kind="ExternalInput").ap()
    qcm = nc.dram_tensor("qcm", [128, sq], f32, kind="ExternalInput").ap()
    kcm = nc.dram_tensor("kcm", [128, skv], f32, kind="ExternalInput").ap()
    wq = nc.dram_tensor("wq", [IN_D, 128], f16, kind="ExternalInput").ap()
    wk = nc.dram_tensor("wk", [IN_D, 128], f16, kind="ExternalInput").ap()
    wv = nc.dram_tensor("wv", [IN_D, 128], f16, kind="ExternalInput").ap()
    wo0 = nc.dram_tensor("wo0", [64, IN_D], f16, kind="ExternalInput").ap()
    wo1 = nc.dram_tensor("wo1", [64, IN_D], f16, kind="ExternalInput").ap()
    ifm = nc.dram_tensor("ifm", [128, 1], f32, kind="ExternalInput").ap()
    sgn = nc.dram_tensor("sgn", [128, 1], f32, kind="ExternalInput").ap()
    out = nc.dram_tensor("out", [sq, IN_D], f32, kind="ExternalOutput").ap()
    dbg_t = {}
    if dbg:
        for nm, shp, dt_ in [("d_ang", [128, skv], f32), ("d_m", [128, skv], f32),
                             ("d_sin", [128, skv], f32), ("d_cos", [128, skv], f32),
                             ("d_krT", [128, skv], f16), ("d_qrT", [128, sq], f16),
                             ("d_vaug", [128, (skv // 128) * 130], f16),
                             ("d_pvn0", [64, sq], f16), ("d_pvn1", [64, sq], f16),
                             ("d_ex", [128, 2 * 512], f16),
                             ("d_kraw", [128, 512], f32),
                             ("d_pv0", [65, 512], f32),
                             ("d_rec", [128, 512], f32),
                             ("d_rbc", [64, 512], f32),
                             ("d_u", [128, 512], f16),
                             ("d_t", [128, 512], f16),
                             ("d_tsw", [128, 512], f16)]:
            dbg_t[nm] = nc.dram_tensor(nm, shp, dt_, kind="ExternalOutput").ap()

    with tile.TileContext(nc) as tc:
        with (
            tc.tile_pool(name="persist", bufs=1) as persist,
            tc.tile_pool(name="wide", bufs=1) as wide,
        ):
            qrT = persist.tile([128, sq], f16)    # rope'd q, feature-major
            krT = persist.tile([128, skv], f16)   # rope'd k, feature-major
            v_aug = persist.tile([128, AO, 130], f16)  # [v0|1|v1|1] token-major
            pvn0 = persist.tile([64, sq], f16)    # normalized attn@v, head 0
            pvn1 = persist.tile([64, sq], f16)
            wq_sb = persist.tile([128, IO, 128], f16)
            wk_sb = persist.tile([128, IO, 128], f16)
            wv_sb = persist.tile([128, IO, 128], f16)
            wo0_sb = persist.tile([64, IN_D], f16)
            wo1_sb = persist.tile([64, IN_D], f16)
            ifm_sb = persist.tile([128, 1], f32)
            sgn_sb = persist.tile([128, 1], f32)
            nbias = persist.tile([128, 1], f32)   # exp shift constant
            nc.vector.memset(nbias, -3.0)

            nc.sync.dma_start(wq_sb, wq.rearrange("(io ii) d -> ii io d", ii=128))
            nc.sync.dma_start(wk_sb, wk.rearrange("(io ii) d -> ii io d", ii=128))
            nc.sync.dma_start(wv_sb, wv.rearrange("(io ii) d -> ii io d", ii=128))
            nc.gpsimd.dma_start(wo0_sb, wo0)
            nc.gpsimd.dma_start(wo1_sb, wo1)
            nc.gpsimd.dma_start(ifm_sb, ifm)
            nc.gpsimd.dma_start(sgn_sb, sgn)
            nc.vector.memset(v_aug[:, :, 64:65], 1.0)
            nc.vector.memset(v_aug[:, :, 129:130], 1.0)

            # ---- Phase 1: load transposed activations, rope tables,
            #      projections + rope, v projection ----
            with (
                tc.tile_pool(name="acts", bufs=1) as acts,
                tc.tile_pool(name="scr", bufs=1) as scr,
                tc.tile_pool(name="rope", bufs=2) as rope,
                tc.tile_pool(name="ps_pr", bufs=2, space="PSUM") as ps_pr,
            ):
                qT = acts.tile([128, IO, sq], f16, tag="qT")
                kvT = acts.tile([128, IO, skv], f16, tag="kvT")
                qx_r = qx.rearrange("s (io ii) -> s io ii", ii=128)
                kvx_r = kvx.rearrange("s (io ii) -> s io ii", ii=128)
                ident = acts.tile([128, 128], f16, tag="ident")
                make_identity(nc, ident)
                for io in range(IO):
                    nc.sync.dma_start(out=kvT[:, io], in_=kvx_r[:, io], transpose=True)
                    nc.sync.dma_start(out=qT[:, io], in_=qx_r[:, io], transpose=True)
                    # HAM warm-up: keep the PE busy while input transposes
                    # stream in, so projections start at full clock.  Each
                    # dummy matmul depends on one transpose, spreading them
                    # across the DMA window.
                    wps = ps_pr.tile([128, 512], f32, tag="warm")
                    nc.tensor.matmul(wps, lhsT=kvT[:, io, 0:128],
                                     rhs=kvT[:, io, 0:512], start=True, stop=True)
                    wps2 = ps_pr.tile([128, 512], f32, tag="warm")
                    nc.tensor.matmul(wps2, lhsT=qT[:, io, 0:128],
                                     rhs=qT[:, io, 0:512], start=True, stop=True)

                for side, (s_len, xT, w_sb, out_rT) in enumerate([
                    (skv, kvT, wk_sb, krT),
                    (sq, qT, wq_sb, qrT),
                ]):
                    coord_dram = kcm if side == 0 else qcm
                    cm = scr.tile([128, s_len], f32, tag=f"cm{side}")
                    nc.gpsimd.dma_start(cm, coord_dram)

                    # angles[p, s] = coord[axis(p), s] * inv_freq(p)
                    ang = scr.tile([128, s_len], f32, tag="ang")
                    nc.vector.tensor_scalar_mul(ang, cm, ifm_sb[:])
                    # range reduction: m = ang - round(ang/2pi)*2pi in [-pi, pi]
                    kq = scr.tile([128, s_len], f32, tag="kq")
                    nc.vector.tensor_scalar(kq, ang, 1.0 / TWO_PI, MAGIC,
                                            Alu.mult, Alu.add)
                    nc.vector.tensor_scalar(kq, kq, MAGIC, None, Alu.subtract)
                    m = scr.tile([128, s_len], f32, tag="m")
                    nc.vector.cody_waite_cascade(m, ang, kq, CW_C1, CW_C2, CW_C3)
                    carg = scr.tile([128, s_len], f32, tag="carg")
                    nc.vector.add_range_wrap(carg, m, math.pi / 2, math.pi, TWO_PI)
                    # sinF[p] = sign(p)*sin(f_p) (sign folded via per-partition
                    # scale), cosF[p] = cos(f_p)
                    sinF = scr.tile([128, s_len], f32, tag="sinF")
                    nc.scalar.activation(sinF, m, AF.Sin, scale=sgn_sb[:, 0:1])
                    cosF = scr.tile([128, s_len], f32, tag="cosF")
                    nc.scalar.activation(cosF, carg, AF.Sin)
                    if dbg and side == 0:
                        nc.gpsimd.dma_start(dbg_t["d_ang"], ang)
                        nc.gpsimd.dma_start(dbg_t["d_m"], m)
                        nc.gpsimd.dma_start(dbg_t["d_sin"], sinF)
                        nc.gpsimd.dma_start(dbg_t["d_cos"], cosF)

                    # projection + rope, 512 columns at a time
                    for blk in range(s_len // 512):
                        raw = ps_pr.tile([128, 512], f32, tag="raw")
                        for io in range(IO):
                            nc.tensor.matmul(raw, lhsT=w_sb[:, io],
                                             rhs=xT[:, io, ts(blk, 512)],
                                             start=(io == 0), stop=(io == IO - 1))
                        u = rope.tile([128, 512], f16, tag="u")
                        nc.vector.tensor_tensor(u, raw, cosF[:, ts(blk, 512)], Alu.mult)
                        t = rope.tile([128, 512], f16, tag="t")
                        nc.vector.tensor_tensor(t, raw, sinF[:, ts(blk, 512)], Alu.mult)
                        tsw = rope.tile([128, 512], f16, tag="tsw")
                        for hb in range(2):     # swap 32-row halves per head
                            lo, hi = hb * 64, hb * 64 + 32
                            nc.gpsimd.dma_start(out=tsw[lo:lo + 32], in_=t[hi:hi + 32])
                            nc.gpsimd.dma_start(out=tsw[hi:hi + 32], in_=t[lo:lo + 32])
                        if dbg and side == 0 and blk == 0:
                            stg_raw = rope.tile([128, 512], f32, tag="dbgraw")
                            nc.vector.tensor_copy(stg_raw, raw)
                            nc.gpsimd.dma_start(dbg_t["d_kraw"], stg_raw)
                            nc.gpsimd.dma_start(dbg_t["d_u"], u)
                            nc.gpsimd.dma_start(dbg_t["d_t"], t)
                            nc.gpsimd.dma_start(dbg_t["d_tsw"], tsw)
                        nc.vector.tensor_tensor(out_rT[:, ts(blk, 512)], u, tsw, Alu.add)

                # v projection: feature-major vT = Wv.T @ kv.T (stationary Wv,
                # N=512 moving), then PE-transpose 128x128 chunks into the
                # token-major augmented V
                for blk in range(skv // 512):
                    vps = ps_pr.tile([128, 512], f32, tag="raw")
                    for io in range(IO):
                        nc.tensor.matmul(vps, lhsT=wv_sb[:, io],
                                         rhs=kvT[:, io, ts(blk, 512)],
                                         start=(io == 0), stop=(io == IO - 1))
                    vT_sb = rope.tile([128, 512], f16, tag="vt")
                    nc.vector.tensor_copy(vT_sb, vps)
                    for t_i in range(4):
                        ao = blk * 4 + t_i
                        vp = ps_pr.tile([128, 128], f16, tag="vp")
                        nc.tensor.transpose(vp, vT_sb[:, ts(t_i, 128)], ident)
                        dst = v_aug[:, ao].rearrange("p (h x) -> p h x", x=65)[:, :, 0:64]
                        nc.vector.tensor_copy(dst, vp.rearrange("p (h d) -> p h d", d=64))

                if dbg:
                    nc.gpsimd.dma_start(dbg_t["d_krT"], krT)
                    nc.gpsimd.dma_start(dbg_t["d_qrT"], qrT)
                    nc.gpsimd.dma_start(dbg_t["d_vaug"],
                                        v_aug.rearrange("p a c -> p (a c)"))

            # ---- Phase 2: attention (scores^T -> exp -> PV+denominator),
            #      then normalize ----
            with (
                tc.tile_pool(name="ps_sc", bufs=2, space="PSUM") as ps_sc,
                tc.tile_pool(name="ps_pv", bufs=2, space="PSUM") as ps_pv,
                tc.tile_pool(name="att", bufs=4) as att,
                tc.tile_pool(name="nrm", bufs=2) as nrm,
            ):
                for sb in range(sq // SB):
                    pv0 = ps_pv.tile([65, SB], f32, tag="pv0")
                    pv1 = ps_pv.tile([65, SB], f32, tag="pv1")
                    for ao in range(AO):
                        sc = ps_sc.tile([128, 2, 512], f32, tag="sc")
                        nc.tensor.matmul(sc[:, 0], lhsT=krT[0:64, ts(ao, 128)],
                                         rhs=qrT[0:64, ts(sb, SB)],
                                         start=True, stop=True, tile_position=(0, 0))
                        nc.tensor.matmul(sc[:, 1], lhsT=krT[64:128, ts(ao, 128)],
                                         rhs=qrT[64:128, ts(sb, SB)],
                                         start=True, stop=True, tile_position=(64, 0))
                        ex = att.tile([128, 2, 512], f16, tag="ex")
                        nc.scalar.activation(ex, sc, AF.Exp, bias=nbias[:], scale=0.125)
                        if dbg and sb == 0 and ao == 0:
                            nc.gpsimd.dma_start(dbg_t["d_ex"],
                                                ex.rearrange("p a b -> p (a b)"))
                        nc.tensor.matmul(pv0, lhsT=v_aug[:, ao, 0:65], rhs=ex[:, 0],
                                         start=(ao == 0), stop=(ao == AO - 1))
                        nc.tensor.matmul(pv1, lhsT=v_aug[:, ao, 65:130], rhs=ex[:, 1],
                                         start=(ao == 0), stop=(ao == AO - 1))
                    for h, pv, pvn in ((0, pv0, pvn0), (1, pv1, pvn1)):
                        den = nrm.tile([128, SB], f32, tag="den")
                        nc.vector.memset(den, 0.0)
                        nc.vector.tensor_copy(den[64:65], pv[64:65])
                        dbc = nrm.tile([128, SB], f32, tag="dbc")
                        nc.gpsimd.partition_all_reduce(
                            dbc, den, channels=128, reduce_op=bass_isa.ReduceOp.add)
                        rbc = nrm.tile([64, SB], f32, tag="rbc")
                        nc.vector.reciprocal_approx_fast(out=rbc, in_=dbc[0:64])
                        if dbg and sb == 0 and h == 0:
                            stg_pv = nrm.tile([65, SB], f32, tag="dbgpv")
                            nc.vector.tensor_copy(stg_pv, pv)
                            nc.gpsimd.dma_start(dbg_t["d_pv0"], stg_pv)
                            nc.gpsimd.dma_start(dbg_t["d_rec"], den)
                            nc.gpsimd.dma_start(dbg_t["d_rbc"], rbc)
                        nc.vector.tensor_tensor(pvn[:, ts(sb, SB)], pv[0:64], rbc,
                                                Alu.mult)

                # output projection as a tail phase (keeps the attention loop
                # free of PSUM-slot contention; row-split Wo -> partial sums,
                # host adds the pair of cores per batch).  lhsT changes once
                # per psum tile to amortize LDWEIGHTS.
                for g in range(sq // 128):
                    if True:
                        op = ps_sc.tile([128, 2, 512], f32, tag="sc")
                        nc.tensor.matmul(op[:, 0], lhsT=pvn0[:, ts(g, 128)],
                                         rhs=wo0_sb[:, 0:512], start=True, stop=False)
                        nc.tensor.matmul(op[:, 1], lhsT=pvn0[:, ts(g, 128)],
                                         rhs=wo0_sb[:, 512:1024], start=True, stop=False)
                        nc.tensor.matmul(op[:, 0], lhsT=pvn1[:, ts(g, 128)],
                                         rhs=wo1_sb[:, 0:512], start=False, stop=True)
                        nc.tensor.matmul(op[:, 1], lhsT=pvn1[:, ts(g, 128)],
                                         rhs=wo1_sb[:, 512:1024], start=False, stop=True)
                        stg = att.tile([128, 2, 512], f32, tag="stg")
                        nc.any.tensor_copy(stg, op)
                        nc.gpsimd.dma_start(out[ts(g, 128), :],
                                            stg.rearrange("p a b -> p (a b)"))

                if dbg:
                    nc.gpsimd.dma_start(dbg_t["d_pvn0"], pvn0)
                    nc.gpsimd.dma_start(dbg_t["d_pvn1"], pvn1)

    nc.compile()
    return nc


def _get_program(sq, skv, dbg=False):
    key = (sq, skv, dbg)
    if key not in _cache:
        _cache[key] = _build_program(sq, skv, dbg=dbg)
    return _cache[key]


def _host_prep(q, q_coords, kv, kv_coords, Wq, Wk, Wv, Wo, sq, skv):
    q = np.ascontiguousarray(np.asarray(q, dtype=np.float32))
    kv = np.ascontiguousarray(np.asarray(kv, dtype=np.float32))
    qc = np.asarray(q_coords).astype(np.float32)
    kc = np.asarray(kv_coords).astype(np.float32)
    Wq = np.asarray(Wq, dtype=np.float32)
    Wk = np.asarray(Wk, dtype=np.float32)
    Wv = np.asarray(Wv, dtype=np.float32)
    Wo = np.asarray(Wo, dtype=np.float32)

    new_order = np.concatenate([np.arange(0, HD, 2), np.arange(1, HD, 2)])
    Wq_eff = Wq.reshape(IN_D, H_EFF, G, HD).sum(axis=2)
    Wq_p = Wq_eff[:, :, new_order].astype(BF16)   # [IN, 4, 64]
    Wk_p = Wk[:, :, new_order].astype(BF16)
    Wv16 = Wv.astype(BF16)
    Wo16 = Wo.astype(BF16)                         # [256, 1024]
    q16 = q.astype(BF16)
    kv16 = kv.astype(BF16)

    inv_freq = (10000.0 ** (-np.arange(16, dtype=np.float64) * (2.0 / 32))
                ).astype(np.float32)
    p = np.arange(128)
    f = (p % 64) % 32
    axis_of_p = f // 16
    j = f % 16
    ifm = inv_freq[j].astype(np.float32).reshape(128, 1)
    sgn = np.where((p % 64) < 32, 1.0, -1.0).astype(np.float32).reshape(128, 1)

    in_maps = []
    for core in range(N_CORES):
        b, hp = core // 2, core % 2
        in_maps.append({
            "qx": np.ascontiguousarray(q16[b, :sq]),
            "kvx": np.ascontiguousarray(kv16[b, :skv]),
            # coordinate of the rope axis used by each partition row
            "qcm": np.ascontiguousarray(qc[b, :sq][:, axis_of_p].T),
            "kcm": np.ascontiguousarray(kc[b, :skv][:, axis_of_p].T),
            "wq": np.ascontiguousarray(Wq_p[:, 2 * hp:2 * hp + 2].reshape(IN_D, 128)),
            "wk": np.ascontiguousarray(Wk_p[:, 2 * hp:2 * hp + 2].reshape(IN_D, 128)),
            "wv": np.ascontiguousarray(Wv16[:, 2 * hp:2 * hp + 2].reshape(IN_D, 128)),
            "wo0": np.ascontiguousarray(Wo16[hp * 128:hp * 128 + 64]),
            "wo1": np.ascontiguousarray(Wo16[hp * 128 + 64:hp * 128 + 128]),
            "ifm": ifm,
            "sgn": sgn,
        })
    return in_maps


def run_sharded(q, q_coords, kv, kv_coords, Wq, Wk, Wv, Wo, sq=S, skv=S,
                trace=False, dbg=False):
    from concourse import bass_utils

    nc = _get_program(sq, skv, dbg=dbg)
    in_maps = _host_prep(q, q_coords, kv, kv_coords, Wq, Wk, Wv, Wo, sq, skv)
    res = bass_utils.run_bass_kernel_spmd(
        nc, in_maps, core_ids=list(range(N_CORES)), trace=trace)
    parts = [r["out"] for r in res.results]
    out = np.stack([parts[2 * b] + parts[2 * b + 1] for b in range(B)])
    return out.astype(np.float32), res


def kernel(q, q_coords, kv, kv_coords, Wq, Wk, Wv, Wo):
    out, _ = run_sharded(q, q_coords, kv, kv_coords, Wq, Wk, Wv, Wo)
    return out
